# revision 1
# baseline (speedup 1.0000x reference)
"""Trainium2 Bass kernel for CorpusSupportSets RBF tangent-field.

Math per sample row i (dim 768), with one-hot mask selecting dipole k:
    k    = argmax(mask[i])            (exact: dot with iota row)
    s0,s1 = SUPPORT_SETS[k] halves;  a_j = ALPHAS[k,j];  g_j = exp(LOGGAMMA[k,j])
    zz = |z|^2, ss_j = |s_j|^2, t_j = z.s_j, n_j = zz - 2 t_j + ss_j
    m_j = a_j * g_j * exp(-g_j n_j)
    beta = (m0+m1)(zz-1) - m0 t0 - m1 t1
    p    = beta z + m0 s0 + m1 s1     (= -proj/2 of the reference, sign-safe)
    out  = p / |p|

Sharding: data-parallel over batch across 8 cores (2048 rows each).
The one-hot matmuls of the reference are replaced by an exact index
computation + indirect-DMA row gather from a host-concatenated table
[SUPPORT_SETS | ALPHAS | LOGGAMMA] of shape [1000, 1540].
"""
import sys

for _p in ("/opt/trn_rl_repo",):
    if _p not in sys.path:
        sys.path.insert(0, _p)

import numpy as np

import concourse.bass as bass
import concourse.tile as tile
from concourse import mybir
from concourse.bass import IndirectOffsetOnAxis
from concourse.bass_utils import run_bass_kernel_spmd
from concourse.vector_clock import ScopedClock

# ---------------------------------------------------------------------------
# Workaround: this walrus build only accepts ONE semaphore wait per
# instruction; the TileContext exit drain accumulates one wait per live
# semaphore lane.  Split overflow waits onto trailing sync-engine NOPs.
_MAX_WAITS = 1


def _split_waits(nc, inst):
    si = inst.sync_info
    if si is None:
        return
    waits = list(si.on_wait)
    if len(waits) <= _MAX_WAITS:
        return
    inst.sync_info = mybir.SyncInfo(
        on_wait=waits[:_MAX_WAITS], on_update=list(si.on_update)
    )
    for i in range(_MAX_WAITS, len(waits), _MAX_WAITS):
        nop = nc.sync.nop(nofuse=True, hint="drain_wait_overflow")
        nop.ins.sync_info = mybir.SyncInfo(
            on_wait=waits[i : i + _MAX_WAITS], on_update=[]
        )


def _patched_drain_and_barrier(self, tick_clock, wait_clock):
    drain_inst = self.nc.sync.drain()
    wait_clock.add_sem_waits(
        drain_inst.ins, ScopedClock({None: tick_clock.global_clock})
    )
    _split_waits(self.nc, drain_inst.ins)
    self.nc.all_engine_barrier()
    assert self.sems is not None
    popped = self.nc._tile_sem_poison_stack.pop()
    assert popped is self._sem_poison
    self.nc.clear_and_free_semaphores(list(self.sems.allocated().values()))
    self.nc.all_engine_barrier()


_orig_commit = tile.TileContext._commit_instruction


def _patched_commit(self, inst, lazy_reg_writes=True):
    si = getattr(inst, "sync_info", None)
    if (
        si is not None
        and si.on_wait
        and len(si.on_wait) > _MAX_WAITS
        and inst.engine != mybir.EngineType.Unassigned
    ):
        waits = list(si.on_wait)
        inst.sync_info = mybir.SyncInfo(
            on_wait=waits[:_MAX_WAITS], on_update=list(si.on_update)
        )
        for _i, _w in enumerate(waits[_MAX_WAITS:]):
            nop = mybir.InstNoOp(
                name=f"{inst.name}_w{_i}",
                engine=inst.engine,
                sync_info=mybir.SyncInfo(on_wait=[_w], on_update=[]),
                bass_nofuse=True,
            )
            self._add_instruction(nop)
    return _orig_commit(self, inst, lazy_reg_writes)


tile.TileContext._drain_and_barrier = _patched_drain_and_barrier
tile.TileContext._commit_instruction = _patched_commit

# ---------------------------------------------------------------------------
BS, K, DIM = 16384, 1000, 768
NCORES = 8
ROWS = BS // NCORES  # 2048 rows per core
P = 128
NT = ROWS // P  # 16 tiles of 128 rows
GRP = 4  # tiles per group
NG = NT // GRP  # 4 groups
TBL_W = 2 * DIM + 4  # 1540: [s0 | s1 | a0 a1 lg0 lg1]
F32 = mybir.dt.float32
U32 = mybir.dt.uint32


def build_nc(rows=ROWS):
    NT = rows // P
    NG = NT // GRP
    OP = mybir.AluOpType
    AT = mybir.ActivationFunctionType
    BF16 = mybir.dt.bfloat16
    nc = bass.Bass()
    zin = nc.dram_tensor("zin", [rows, DIM], F32, kind="ExternalInput")
    mk = nc.dram_tensor("mk", [rows, K], BF16, kind="ExternalInput")
    tbl = nc.dram_tensor("tbl", [K, TBL_W], F32, kind="ExternalInput")
    out = nc.dram_tensor("out", [rows, DIM], F32, kind="ExternalOutput")

    with tile.TileContext(nc) as tc:
        with (
            tc.tile_pool(name="zp", bufs=3) as zp,
            tc.tile_pool(name="mkp", bufs=2) as mkp,
            tc.tile_pool(name="selp", bufs=3) as selp,
            tc.tile_pool(name="outp", bufs=2) as outp,
            tc.tile_pool(name="scrD", bufs=4, space="PSUM") as scrDp,
            tc.tile_pool(name="wp", bufs=4) as wp,
            tc.tile_pool(name="tiny", bufs=40) as tinyp,
            tc.tile_pool(name="singles", bufs=1) as singles,
        ):
            ss0a = singles.tile([P, NT], F32)
            ss1a = singles.tile([P, NT], F32)
            q0a = singles.tile([P, NT], F32)
            q1a = singles.tile([P, NT], F32)
            pna = singles.tile([P, NT], F32)
            sqa = singles.tile([P, NT], F32)
            ra = singles.tile([P, NT], F32)
            sidea = singles.tile([P, NT, 4], F32)
            mia = singles.tile([P, NT, 8], U32)

            def phase1(g):
                r0, r1 = g * GRP * P, (g + 1) * GRP * P
                c0, c1 = g * GRP, (g + 1) * GRP
                z_g = zp.tile([P, GRP, DIM], F32, name="z_g", tag="z")
                nc.sync.dma_start(
                    out=z_g[:], in_=zin[r0:r1].rearrange("(n p) c -> p n c", p=P)
                )
                mk_g = mkp.tile([P, GRP, K], mybir.dt.bfloat16, name="mk_g", tag="mk")
                nc.sync.dma_start(
                    out=mk_g[:], in_=mk[r0:r1].rearrange("(n p) c -> p n c", p=P)
                )
                # argmax of one-hot mask, gather table rows into one tile
                sel4 = selp.tile([P, GRP, TBL_W], F32, name="sel4", tag="sel")
                for n in range(GRP):
                    j = c0 + n
                    mx = tinyp.tile([P, 8], mybir.dt.bfloat16, name="mx", tag="mx")
                    nc.vector.max(out=mx[:], in_=mk_g[:, n, :])
                    nc.vector.max_index(
                        out=mia[:, j, :], in_max=mx[:], in_values=mk_g[:, n, :]
                    )
                    nc.gpsimd.indirect_dma_start(
                        out=sel4[:, n, :],
                        out_offset=None,
                        in_=tbl[:],
                        in_offset=IndirectOffsetOnAxis(ap=mia[:, j, 0:1], axis=0),
                    )
                # batched w = z + s for both poles (single big DVE ops)
                w0 = wp.tile([P, GRP, DIM], F32, name="w0", tag="w")
                nc.vector.tensor_tensor(
                    out=w0[:], in0=z_g[:], in1=sel4[:, :, :DIM], op=OP.add
                )
                w1 = wp.tile([P, GRP, DIM], F32, name="w1", tag="w")
                nc.vector.tensor_tensor(
                    out=w1[:], in0=z_g[:], in1=sel4[:, :, DIM : 2 * DIM], op=OP.add
                )
                # per-row reductions on ACT (accumulate along free axis)
                for n in range(GRP):
                    j = c0 + n
                    nc.scalar.activation(
                        out=scrDp.tile([P, DIM], F32, name="scrd", tag="scrD")[:],
                        in_=sel4[:, n, :DIM], func=AT.Square,
                        accum_out=ss0a[:, j : j + 1],
                    )
                    nc.scalar.activation(
                        out=scrDp.tile([P, DIM], F32, name="scrd", tag="scrD")[:],
                        in_=sel4[:, n, DIM : 2 * DIM], func=AT.Square,
                        accum_out=ss1a[:, j : j + 1],
                    )
                    nc.scalar.activation(
                        out=scrDp.tile([P, DIM], F32, name="scrd", tag="scrD")[:],
                        in_=w0[:, n, :], func=AT.Square, accum_out=q0a[:, j : j + 1],
                    )
                    nc.scalar.activation(
                        out=scrDp.tile([P, DIM], F32, name="scrd", tag="scrD")[:],
                        in_=w1[:, n, :], func=AT.Square, accum_out=q1a[:, j : j + 1],
                    )
                nc.gpsimd.tensor_copy(
                    out=sidea[:, c0:c1, :], in_=sel4[:, :, 2 * DIM :]
                )

                # per-group small math on [P, GRP] columns
                def _m(qv, ssv, av, lgv, eng):
                    gt = tinyp.tile([P, GRP], F32, name="gt", tag="tiny")
                    nc.scalar.activation(out=gt[:], in_=lgv, func=AT.Exp)
                    d = tinyp.tile([P, GRP], F32, name="d", tag="tiny")
                    eng.tensor_scalar(
                        out=d[:], in0=ssv, scalar1=1.0, scalar2=None, op0=OP.add
                    )
                    t2 = tinyp.tile([P, GRP], F32, name="t2", tag="tiny")
                    eng.tensor_tensor(out=t2[:], in0=qv, in1=d[:], op=OP.subtract)
                    nn = tinyp.tile([P, GRP], F32, name="nn", tag="tiny")
                    eng.tensor_scalar(
                        out=nn[:], in0=d[:], scalar1=2.0, scalar2=None, op0=OP.mult
                    )
                    eng.tensor_tensor(out=nn[:], in0=nn[:], in1=qv, op=OP.subtract)
                    eng.tensor_tensor(out=nn[:], in0=nn[:], in1=gt[:], op=OP.mult)
                    e = tinyp.tile([P, GRP], F32, name="e", tag="tiny")
                    nc.scalar.activation(out=e[:], in_=nn[:], func=AT.Exp, scale=-1.0)
                    m = tinyp.tile([P, GRP], F32, name="m", tag="tiny")
                    eng.tensor_tensor(out=m[:], in0=e[:], in1=gt[:], op=OP.mult)
                    eng.tensor_tensor(out=m[:], in0=m[:], in1=av, op=OP.mult)
                    return m, t2

                m0, t20 = _m(
                    q0a[:, c0:c1], ss0a[:, c0:c1],
                    sidea[:, c0:c1, 0], sidea[:, c0:c1, 2], nc.vector,
                )
                m1, t21 = _m(
                    q1a[:, c0:c1], ss1a[:, c0:c1],
                    sidea[:, c0:c1, 1], sidea[:, c0:c1, 3], nc.gpsimd,
                )
                # beta = -(m0*t20 + m1*t21)/2   (zz == 1)
                h0 = tinyp.tile([P, GRP], F32, name="h0", tag="tiny")
                nc.vector.tensor_tensor(out=h0[:], in0=m0[:], in1=t20[:], op=OP.mult)
                h1 = tinyp.tile([P, GRP], F32, name="h1", tag="tiny")
                nc.gpsimd.tensor_tensor(out=h1[:], in0=m1[:], in1=t21[:], op=OP.mult)
                bB = tinyp.tile([P, GRP], F32, name="bB", tag="tiny")
                nc.vector.tensor_tensor(out=bB[:], in0=h0[:], in1=h1[:], op=OP.add)
                nc.vector.tensor_scalar(
                    out=bB[:], in0=bB[:], scalar1=-0.5, scalar2=None, op0=OP.mult
                )
                return dict(g=g, z_g=z_g, sel4=sel4, m0=m0, m1=m1, bB=bB)

            def phase2(st):
                g = st["g"]
                r0, r1 = g * GRP * P, (g + 1) * GRP * P
                c0, c1 = g * GRP, (g + 1) * GRP
                z_g, sel4, m0, m1, bB = (
                    st["z_g"], st["sel4"], st["m0"], st["m1"], st["bB"]
                )
                pg = outp.tile([P, GRP, DIM], F32, name="pg", tag="pg")
                for n in range(GRP):
                    j = c0 + n
                    p_n = pg[:, n, :]
                    nc.vector.tensor_scalar(
                        out=p_n, in0=z_g[:, n, :], scalar1=bB[:, n : n + 1],
                        scalar2=None, op0=OP.mult,
                    )
                    nc.vector.scalar_tensor_tensor(
                        out=p_n, in0=sel4[:, n, :DIM], scalar=m0[:, n : n + 1],
                        in1=p_n, op0=OP.mult, op1=OP.add,
                    )
                    nc.vector.scalar_tensor_tensor(
                        out=p_n, in0=sel4[:, n, DIM : 2 * DIM],
                        scalar=m1[:, n : n + 1],
                        in1=p_n, op0=OP.mult, op1=OP.add,
                    )
                    nc.scalar.activation(
                        out=scrDp.tile([P, DIM], F32, name="scrd", tag="scrD")[:],
                        in_=p_n, func=AT.Square, accum_out=pna[:, j : j + 1],
                    )
                nc.scalar.activation(
                    out=sqa[:, c0:c1], in_=pna[:, c0:c1], func=AT.Sqrt
                )
                nc.vector.reciprocal(out=ra[:, c0:c1], in_=sqa[:, c0:c1])
                for n in range(GRP):
                    j = c0 + n
                    nc.vector.tensor_scalar(
                        out=pg[:, n, :], in0=pg[:, n, :], scalar1=ra[:, j : j + 1],
                        scalar2=None, op0=OP.mult,
                    )
                nc.sync.dma_start(
                    out=out[r0:r1].rearrange("(n p) c -> p n c", p=P), in_=pg[:]
                )

            pending = None
            for g in range(NG):
                st = phase1(g)
                if pending is not None:
                    phase2(pending)
                pending = st
            phase2(pending)
    return nc


_NC_CACHE = None


def _get_nc():
    global _NC_CACHE
    if _NC_CACHE is None:
        _NC_CACHE = build_nc()
    return _NC_CACHE


def build_in_maps(inputs):
    import ml_dtypes

    z = np.ascontiguousarray(inputs["z"], dtype=np.float32)
    mask = np.asarray(inputs["support_sets_mask"], dtype=np.float32)
    mk = mask.astype(ml_dtypes.bfloat16)
    tbl = np.ascontiguousarray(
        np.concatenate(
            [
                np.asarray(inputs["SUPPORT_SETS"], dtype=np.float32),
                np.asarray(inputs["ALPHAS"], dtype=np.float32),
                np.asarray(inputs["LOGGAMMA"], dtype=np.float32),
            ],
            axis=1,
        )
    )
    return [
        {
            "zin": np.ascontiguousarray(z[c * ROWS : (c + 1) * ROWS]),
            "mk": np.ascontiguousarray(mk[c * ROWS : (c + 1) * ROWS]),
            "tbl": tbl,
        }
        for c in range(NCORES)
    ]


def kernel(support_sets_mask, z, SUPPORT_SETS, ALPHAS, LOGGAMMA):
    in_maps = build_in_maps(
        dict(
            support_sets_mask=support_sets_mask, z=z,
            SUPPORT_SETS=SUPPORT_SETS, ALPHAS=ALPHAS, LOGGAMMA=LOGGAMMA,
        )
    )
    nc = _get_nc()
    res = run_bass_kernel_spmd(nc, in_maps, list(range(NCORES)))
    return np.concatenate([res.results[c]["out"] for c in range(NCORES)], axis=0)



# revision 13
# speedup vs baseline: 1.5227x; 1.5227x over previous
"""Trainium2 Bass kernel for CorpusSupportSets RBF tangent-field.

Math per sample row i (dim 768), one-hot mask selects dipole k:
    t_j  = z . s_j                      (unit z, unit s_j)
    m_j  = a_j g_j e^{-g_j(2-2t_j)} = C_j exp(2 g_j t_j),  C_j = a_j g_j e^{-2 g_j}
    beta = -(m0 t0 + m1 t1)
    p    = beta z + m0 s0 + m1 s1
    |p|^2 = m0^2 + m1^2 - beta^2 + 2 m0 m1 (s0.s1)
    out  = p / |p|

Sharding: data-parallel over batch across 8 cores (2048 rows each).
Host prep (dtype/layout only + per-table-row constants): z -> bf16;
mask -> f16 scaled by column index (one-hot * k, exact in f16); table
rows [s0|s1|C0|C1|2g0|2g1|c01|pad] in bf16 (1664 cols = 3328B, 256B
multiple for dma_gather); output computed in bf16, upcast on host.

Per-sample row selection uses indirect DMA row gathers with u32
per-partition offsets computed from the scaled-mask reduction.
"""
import sys

for _p in ("/opt/trn_rl_repo",):
    if _p not in sys.path:
        sys.path.insert(0, _p)

import numpy as np

import concourse.bass as bass
import concourse.tile as tile
from concourse import mybir
from concourse.bass_utils import run_bass_kernel_spmd
from concourse.vector_clock import ScopedClock

# ---------------------------------------------------------------------------
# Workaround: this walrus build only accepts ONE semaphore wait per
# instruction; the TileContext exit drain accumulates one wait per live
# semaphore lane.  Split overflow waits onto trailing sync-engine NOPs.
_MAX_WAITS = 1


def _split_waits(nc, inst):
    si = inst.sync_info
    if si is None:
        return
    waits = list(si.on_wait)
    if len(waits) <= _MAX_WAITS:
        return
    inst.sync_info = mybir.SyncInfo(
        on_wait=waits[:_MAX_WAITS], on_update=list(si.on_update)
    )
    for i in range(_MAX_WAITS, len(waits), _MAX_WAITS):
        nop = nc.sync.nop(nofuse=True, hint="drain_wait_overflow")
        nop.ins.sync_info = mybir.SyncInfo(
            on_wait=waits[i : i + _MAX_WAITS], on_update=[]
        )


def _patched_drain_and_barrier(self, tick_clock, wait_clock):
    drain_inst = self.nc.sync.drain()
    wait_clock.add_sem_waits(
        drain_inst.ins, ScopedClock({None: tick_clock.global_clock})
    )
    _split_waits(self.nc, drain_inst.ins)
    self.nc.all_engine_barrier()
    assert self.sems is not None
    popped = self.nc._tile_sem_poison_stack.pop()
    assert popped is self._sem_poison
    self.nc.clear_and_free_semaphores(list(self.sems.allocated().values()))
    self.nc.all_engine_barrier()


_orig_commit = tile.TileContext._commit_instruction


def _patched_commit(self, inst, lazy_reg_writes=True):
    si = getattr(inst, "sync_info", None)
    if (
        si is not None
        and si.on_wait
        and len(si.on_wait) > _MAX_WAITS
        and inst.engine != mybir.EngineType.Unassigned
    ):
        waits = list(si.on_wait)
        inst.sync_info = mybir.SyncInfo(
            on_wait=waits[:_MAX_WAITS], on_update=list(si.on_update)
        )
        for _i, _w in enumerate(waits[_MAX_WAITS:]):
            nop = mybir.InstNoOp(
                name=f"{inst.name}_w{_i}",
                engine=inst.engine,
                sync_info=mybir.SyncInfo(on_wait=[_w], on_update=[]),
                bass_nofuse=True,
            )
            self._add_instruction(nop)
    return _orig_commit(self, inst, lazy_reg_writes)


tile.TileContext._drain_and_barrier = _patched_drain_and_barrier
tile.TileContext._commit_instruction = _patched_commit

# ---------------------------------------------------------------------------
BS, K, DIM = 16384, 1000, 768
NCORES = 8
ROWS = BS // NCORES  # 2048 rows per core
P = 128
NT = ROWS // P  # 16 tiles of 128 rows
GRP = 4  # tiles per group
NG = NT // GRP  # 4 groups
TBL_W = 2 * DIM + 8  # 1544 bf16 cols = 3088B per row
# param columns inside a table row
PC = 2 * DIM  # C0, C1, 2g0, 2g1, c01 start here
F32 = mybir.dt.float32
BF16 = mybir.dt.bfloat16
F16 = mybir.dt.float16
U32 = mybir.dt.uint32


def build_nc(rows=ROWS):
    NT = rows // P
    NG = NT // GRP
    OP = mybir.AluOpType
    AT = mybir.ActivationFunctionType
    nc = bass.Bass()
    zin = nc.dram_tensor("zin", [rows, DIM], BF16, kind="ExternalInput")
    mkv = nc.dram_tensor("mkv", [rows, K], F16, kind="ExternalInput")
    tbl = nc.dram_tensor("tbl", [K, TBL_W], BF16, kind="ExternalInput")
    out = nc.dram_tensor("out", [rows, DIM], BF16, kind="ExternalOutput")

    with tile.TileContext(nc) as tc:
        with (
            tc.tile_pool(name="zp", bufs=3) as zp,
            tc.tile_pool(name="mkp", bufs=2) as mkp,
            tc.tile_pool(name="selp", bufs=3) as selp,
            tc.tile_pool(name="outp", bufs=2) as outp,
            tc.tile_pool(name="scrap", bufs=4) as scrp,
            tc.tile_pool(name="ascrap", bufs=2) as ascrp,
            tc.tile_pool(name="tiny", bufs=48) as tinyp,
            tc.tile_pool(name="singles", bufs=1) as singles,
        ):
            # per-sample accumulators, one column per (tile, pole)
            t_a = singles.tile([P, NT * 2], F32)  # t0,t1 interleaved per tile
            idxf = singles.tile([P, NT], F32)  # idx as f32
            idxu = singles.tile([P, NT], U32)  # idx as u32 gather offsets

            def phase1(g):
                r0, r1 = g * GRP * P, (g + 1) * GRP * P
                c0 = g * GRP
                z_g = zp.tile([P, GRP, DIM], BF16, name="z_g", tag="z")
                nc.sync.dma_start(
                    out=z_g[:], in_=zin[r0:r1].rearrange("(n p) c -> p n c", p=P)
                )
                mk_g = mkp.tile([P, GRP, K], F16, name="mk_g", tag="mk")
                nc.sync.dma_start(
                    out=mk_g[:], in_=mkv[r0:r1].rearrange("(n p) c -> p n c", p=P)
                )
                # idx per tile: free-axis sum of scaled one-hot (exact)
                for n in range(GRP):
                    j = c0 + n
                    if n % 2 == 0:
                        nc.vector.tensor_reduce(
                            out=idxf[:, j : j + 1],
                            in_=mk_g[:, n, :],
                            axis=mybir.AxisListType.X,
                            op=OP.add,
                        )
                    else:
                        a_s = ascrp.tile([P, K], F16, name="a_s", tag="ascr")
                        nc.scalar.activation(
                            out=a_s[:],
                            in_=mk_g[:, n, :],
                            func=AT.Copy,
                            accum_out=idxf[:, j : j + 1],
                        )
                # convert group idx to u32 gather offsets
                nc.gpsimd.tensor_copy(
                    out=idxu[:, c0 : c0 + GRP], in_=idxf[:, c0 : c0 + GRP]
                )
                # gather table rows (one indirect DMA per tile)
                sel = selp.tile([P, GRP, TBL_W], BF16, name="sel", tag="sel")
                for n in range(GRP):
                    j = c0 + n
                    nc.gpsimd.indirect_dma_start(
                        out=sel[:, n, :],
                        out_offset=None,
                        in_=tbl[:],
                        in_offset=bass.IndirectOffsetOnAxis(
                            ap=idxu[:, j : j + 1], axis=0
                        ),
                    )
                # t_j = z . s_j: one DVE op (elementwise mult + free-axis accum)
                for n in range(GRP):
                    j = c0 + n
                    for pole in range(2):
                        pscr = scrp.tile([P, DIM], BF16, name="pscr", tag="scr")
                        nc.vector.scalar_tensor_tensor(
                            out=pscr[:],
                            in0=z_g[:, n, :],
                            scalar=1.0,
                            in1=sel[:, n, pole * DIM : (pole + 1) * DIM],
                            op0=OP.mult,
                            op1=OP.mult,
                            accum_out=t_a[:, 2 * j + pole : 2 * j + pole + 1],
                        )
                return dict(g=g, z_g=z_g, sel=sel)

            def phase2(st):
                g = st["g"]
                r0, r1 = g * GRP * P, (g + 1) * GRP * P
                c0 = g * GRP
                z_g, sel = st["z_g"], st["sel"]
                tg = t_a[:, 8 * g : 8 * (g + 1)].rearrange(
                    "p (c t) -> p c t", t=2
                )  # [P, GRP, 2] f32
                # small per-sample math on [P, GRP(,2)] tiles
                selC = sel[:, :, PC : PC + 2]  # bf16 [P,GRP,2]
                selG2 = sel[:, :, PC + 2 : PC + 4]
                selc01 = sel[:, :, PC + 4 : PC + 5]
                Cf = tinyp.tile([P, GRP, 2], F32, name="Cf", tag="tiny")
                nc.gpsimd.tensor_copy(out=Cf[:], in_=selC)
                G2f = tinyp.tile([P, GRP, 2], F32, name="G2f", tag="tiny")
                nc.gpsimd.tensor_copy(out=G2f[:], in_=selG2)
                c01f = tinyp.tile([P, GRP], F32, name="c01f", tag="tiny")
                nc.gpsimd.tensor_copy(
                    out=c01f[:], in_=selc01.rearrange("p c o -> p (c o)")
                )
                u = tinyp.tile([P, GRP, 2], F32, name="u", tag="tiny")
                nc.vector.tensor_tensor(out=u[:], in0=G2f[:], in1=tg, op=OP.mult)
                e = tinyp.tile([P, GRP, 2], F32, name="e", tag="tiny")
                nc.scalar.activation(out=e[:], in_=u[:], func=AT.Exp)
                m = tinyp.tile([P, GRP, 2], F32, name="m", tag="tiny")
                nc.vector.tensor_tensor(out=m[:], in0=Cf[:], in1=e[:], op=OP.mult)
                h = tinyp.tile([P, GRP, 2], F32, name="h", tag="tiny")
                nc.vector.tensor_tensor(out=h[:], in0=m[:], in1=tg, op=OP.mult)
                beta = tinyp.tile([P, GRP], F32, name="beta", tag="tiny")
                nc.vector.scalar_tensor_tensor(
                    out=beta[:], in0=h[:, :, 0], scalar=-1.0, in1=h[:, :, 1],
                    op0=OP.mult, op1=OP.subtract,
                )
                m2 = tinyp.tile([P, GRP, 2], F32, name="m2", tag="tiny")
                nc.vector.tensor_tensor(out=m2[:], in0=m[:], in1=m[:], op=OP.mult)
                s2 = tinyp.tile([P, GRP], F32, name="s2", tag="tiny")
                nc.vector.tensor_tensor(
                    out=s2[:], in0=m2[:, :, 0], in1=m2[:, :, 1], op=OP.add
                )
                mm = tinyp.tile([P, GRP], F32, name="mm", tag="tiny")
                nc.vector.tensor_tensor(
                    out=mm[:], in0=m[:, :, 0], in1=m[:, :, 1], op=OP.mult
                )
                v = tinyp.tile([P, GRP], F32, name="v", tag="tiny")
                nc.vector.tensor_tensor(out=v[:], in0=mm[:], in1=c01f[:], op=OP.mult)
                b2 = tinyp.tile([P, GRP], F32, name="b2", tag="tiny")
                nc.vector.tensor_tensor(out=b2[:], in0=beta[:], in1=beta[:], op=OP.mult)
                w = tinyp.tile([P, GRP], F32, name="w", tag="tiny")
                nc.vector.scalar_tensor_tensor(
                    out=w[:], in0=v[:], scalar=2.0, in1=b2[:],
                    op0=OP.mult, op1=OP.subtract,
                )
                pn = tinyp.tile([P, GRP], F32, name="pn", tag="tiny")
                nc.vector.tensor_tensor(out=pn[:], in0=s2[:], in1=w[:], op=OP.add)
                sq = tinyp.tile([P, GRP], F32, name="sq", tag="tiny")
                nc.scalar.activation(out=sq[:], in_=pn[:], func=AT.Sqrt)
                rr = tinyp.tile([P, GRP], F32, name="rr", tag="tiny")
                nc.vector.reciprocal(out=rr[:], in_=sq[:])
                bp = tinyp.tile([P, GRP], F32, name="bp", tag="tiny")
                nc.vector.tensor_tensor(out=bp[:], in0=beta[:], in1=rr[:], op=OP.mult)
                m0p = tinyp.tile([P, GRP], F32, name="m0p", tag="tiny")
                nc.vector.tensor_tensor(
                    out=m0p[:], in0=m[:, :, 0], in1=rr[:], op=OP.mult
                )
                m1p = tinyp.tile([P, GRP], F32, name="m1p", tag="tiny")
                nc.vector.tensor_tensor(
                    out=m1p[:], in0=m[:, :, 1], in1=rr[:], op=OP.mult
                )
                # p = bp*z + m0p*s0 + m1p*s1   (ACT -> DVE -> GpSimd chain)
                og = outp.tile([P, GRP, DIM], BF16, name="og", tag="og")
                for n in range(GRP):
                    p1 = scrp.tile([P, DIM], BF16, name="p1", tag="scr")
                    nc.scalar.activation(
                        out=p1[:], in_=z_g[:, n, :], func=AT.Copy,
                        scale=bp[:, n : n + 1],
                    )
                    nc.vector.scalar_tensor_tensor(
                        out=p1[:], in0=sel[:, n, 0:DIM], scalar=m0p[:, n : n + 1],
                        in1=p1[:], op0=OP.mult, op1=OP.add,
                    )
                    nc.vector.scalar_tensor_tensor(
                        out=og[:, n, :], in0=sel[:, n, DIM : 2 * DIM],
                        scalar=m1p[:, n : n + 1],
                        in1=p1[:], op0=OP.mult, op1=OP.add,
                    )
                nc.sync.dma_start(
                    out=out[r0:r1].rearrange("(n p) c -> p n c", p=P), in_=og[:]
                )

            pending = None
            for g in range(NG):
                st = phase1(g)
                if pending is not None:
                    phase2(pending)
                pending = st
            phase2(pending)
    return nc


_NC_CACHE = None


def _get_nc():
    global _NC_CACHE
    if _NC_CACHE is None:
        _NC_CACHE = build_nc()
    return _NC_CACHE


def build_in_maps(inputs):
    import ml_dtypes

    z = np.asarray(inputs["z"], dtype=np.float32)
    mask = np.asarray(inputs["support_sets_mask"], dtype=np.float32)
    S = np.asarray(inputs["SUPPORT_SETS"], dtype=np.float32)
    A = np.asarray(inputs["ALPHAS"], dtype=np.float32)
    LG = np.asarray(inputs["LOGGAMMA"], dtype=np.float32)

    zb = z.astype(ml_dtypes.bfloat16)
    # scaled one-hot: value = column index (exact in f16 for k < 2048)
    mkv = (mask * np.arange(K, dtype=np.float32)[None, :]).astype(np.float16)
    # table rows: [s0 | s1 | C0 C1 2g0 2g1 c01 | pad]
    g = np.exp(LG)  # [K,2]
    C = A * g * np.exp(-2.0 * g)
    c01 = np.sum(S[:, :DIM] * S[:, DIM:], axis=1, keepdims=True)
    tblf = np.zeros((K, TBL_W), dtype=np.float32)
    tblf[:, : 2 * DIM] = S
    tblf[:, PC : PC + 2] = C
    tblf[:, PC + 2 : PC + 4] = 2.0 * g
    tblf[:, PC + 4 : PC + 5] = c01
    tbl = tblf.astype(ml_dtypes.bfloat16)

    return [
        {
            "zin": np.ascontiguousarray(zb[c * ROWS : (c + 1) * ROWS]),
            "mkv": np.ascontiguousarray(mkv[c * ROWS : (c + 1) * ROWS]),
            "tbl": tbl,
        }
        for c in range(NCORES)
    ]


def kernel(support_sets_mask, z, SUPPORT_SETS, ALPHAS, LOGGAMMA):
    in_maps = build_in_maps(
        dict(
            support_sets_mask=support_sets_mask, z=z,
            SUPPORT_SETS=SUPPORT_SETS, ALPHAS=ALPHAS, LOGGAMMA=LOGGAMMA,
        )
    )
    nc = _get_nc()
    res = run_bass_kernel_spmd(nc, in_maps, list(range(NCORES)))
    return np.concatenate(
        [res.results[c]["out"] for c in range(NCORES)], axis=0
    ).astype(np.float32)
